# revision 2
# baseline (speedup 1.0000x reference)
"""AttentionPooling (global-softmax segment-sum) Trainium2 Bass kernel, v2.

  scores = x @ W + b ; attn = softmax(scores, axis=0) ; out = segment_sum(x*attn, batch, G)

Design (8 cores, SPMD, raw Bass; one fused semaphore wait per instruction):

 * softmax shift-invariant => b drops out; fixed shift 0 (scores ~ N(0,1)).
 * host streams xw = x * W^T (bf16). Device computes per core the
   unnormalized pooledT'[d, g] = sum_{i in g} e^{s_i} xw[i, d] and
   Z_core = sum_i e^{s_i}; host divides by W[d] (undoes the fold) and by
   Z = sum Z_core. scores s_i = sum_d xw[i, d] (device fold tree + reduce).
 * shard by SEGMENT BLOCKS of SEGW=32 segs: G -> cores x blocks x 32. batch
   sorted => block nodes contiguous; host pads every block to blk_ch chunks
   of 128 nodes. Pad nodes: xw=0 => e^0=1 pollutes only Z (host subtracts
   the pad count); pad batchloc=999 never matches any one-hot column.
 * per block b (one x-tile [128, blk_ch x 128] bf16):
     SYNC dma xt[b%NXB]
     DVE  fold tree d:128->64->32->16->8 (bf16 TT, 2x mode) into xw scratch,
          then 1x tensor_reduce -> scores[:, blk] f32
     ACT  expw = Exp(scores) with accum_out -> zc[:, b]
     one-hot A[p, j] = (iota_j == batchloc_p) * expw_p  [128 nodes, 32 segs],
     built per chunk, split kd on DVE (TS is_eq+mult, 4x, ~70ns), kg on Pool
     (~139ns), ka on ACT (Square(iota+bln) then Exp(-100*u + s), exact at
     integer iota/bl; ~424ns)
     PE   psumT[128 d, 32 segs] += xt_chunk.T @ A  (lhsT=xt chunk, rhs=A;
          out free = 32 => 13-27ns/chunk at full speed)
   one-hot work of block b-1 overlaps scores of block b (software pipeline).
 * psum: 2 banks [128, 512]; 16 blocks (512 segs) per bank; ACT copies each
   full bank -> bf16 stage; one DMA out per bank group.
 * waits are FUSED onto compute instructions (wait queue, not SEQ) wherever
   only one condition is needed; separate wait_ge otherwise.
"""

import os
import numpy as np
import ml_dtypes

import concourse.bass as bass
import concourse.mybir as mybir
from concourse.bass_utils import run_bass_kernel_spmd

BF16 = mybir.dt.bfloat16
F32 = mybir.dt.float32
ALU = mybir.AluOpType
ACTF = mybir.ActivationFunctionType

N_CORES = 8
D = 128
P = 128
SEGW = 32              # segments per block (= one-hot width = psum slice)
BPB = 16               # blocks per psum bank group (16*32 = 512 f32 cols)
NXB = 12               # x-tile buffer depth
NAT = 16               # one-hot tile slots per producing engine
BIGM = -100.0          # ACT one-hot: A = Exp(BIGM*u + s), u=(iota-bl)^2

_prog_cache = {}


def _build(blocks, blk_ch, kd, kg):
    """blocks SEGW-seg blocks/core; blk_ch chunks (of 128 nodes) per block;
    one-hot split per block: kd chunks on DVE, kg on Pool, rest on ACT."""
    ka = blk_ch - kd - kg
    assert ka >= 0
    fusew = int(os.environ.get("FUSEW", "1"))
    nch = blocks * blk_ch
    ngrp = (blocks + BPB - 1) // BPB

    nc = bass.Bass()

    xp_h = nc.declare_dram_parameter("xp", [nch * P * D], BF16, isOutput=False)
    bl_h = nc.declare_dram_parameter("bl", [P, nch], F32, isOutput=False)
    iota_h = nc.declare_dram_parameter("iota", [P, SEGW], BF16, isOutput=False)
    out_h = nc.declare_dram_parameter("outp", [P, blocks * SEGW], BF16, isOutput=True)
    z_h = nc.declare_dram_parameter("zout", [P, 1], F32, isOutput=True)
    dbg = int(os.environ.get("DBG", "0"))
    if dbg:
        sc_h = nc.declare_dram_parameter("scdbg", [P, nch], F32, isOutput=True)
        ew_h = nc.declare_dram_parameter("ewdbg", [P, nch], F32, isOutput=True)

    # tick tables (pure counting in emission order) --------------------------
    # DVE: bln(1); per b: F1..F4, RED (sem-chained); per b>=1: kd one-hots;
    # final zred
    T_DVE_F1, T_DVE_F = {}, {}
    T_DVE_RED, T_DVE_OH = {}, {}
    t = 1  # bln
    for b in range(blocks + 1):
        if b < blocks:
            t += 1; T_DVE_F1[b] = t
            for k in range(3):
                t += 1; T_DVE_F[(b, k)] = t
            t += 1; T_DVE_RED[b] = t
        if b >= 1:
            for i in range(kd):
                t += 1; T_DVE_OH[(b - 1, i)] = t
    zred_tick = t + 1
    # Pool: per b>=1: kg one-hots
    T_GP_OH = {}
    t = 0
    for b in range(1, blocks + 1):
        for i in range(kg):
            t += 1; T_GP_OH[(b - 1, i)] = t
    # ACT: per b: Exp(+1); per b>=1: ka pairs (+1 on the Exp of each pair)
    T_ACT_EXP, T_ACT_OH = {}, {}
    t = 0
    for b in range(blocks + 1):
        if b < blocks:
            t += 1; T_ACT_EXP[b] = t
        if b >= 1:
            for j in range(ka):
                t += 1; T_ACT_OH[(b - 1, j)] = t
    # PE: one mm per chunk
    T_PE = {}
    t = 0
    for b in range(blocks):
        for c in range(blk_ch):
            t += 1; T_PE[(b, c)] = t

    # chunk position of each engine's i-th one-hot within a block
    def pos_d(i):
        return i

    def pos_g(i):
        return kd + i

    def pos_a(j):
        return kd + kg + j

    import contextlib
    with contextlib.ExitStack() as ctx:
        sem_xc = ctx.enter_context(nc.semaphore("sem_xc"))
        sem_x = [ctx.enter_context(nc.semaphore(f"sem_x{j}")) for j in range(NXB)]
        sem_dve = ctx.enter_context(nc.semaphore("sem_dve"))
        sem_act = ctx.enter_context(nc.semaphore("sem_act"))
        sem_gp = ctx.enter_context(nc.semaphore("sem_gp"))
        sem_pe = ctx.enter_context(nc.semaphore("sem_pe"))
        sem_cp = ctx.enter_context(nc.semaphore("sem_cp"))
        sem_out = ctx.enter_context(nc.semaphore("sem_out"))

        iota_t = ctx.enter_context(nc.sbuf_tensor([P, SEGW], BF16))
        bl_t = ctx.enter_context(nc.sbuf_tensor([P, nch], F32))
        bln_t = ctx.enter_context(nc.sbuf_tensor([P, nch], F32))
        xt = [ctx.enter_context(nc.sbuf_tensor(f"xt{j}", [P, blk_ch * D], BF16))
              for j in range(NXB)]
        xw_t = ctx.enter_context(nc.sbuf_tensor([P, blk_ch * (D // 2)], BF16))
        scores_t = ctx.enter_context(nc.sbuf_tensor([P, nch], F32))
        expw_t = ctx.enter_context(nc.sbuf_tensor([P, nch], F32))
        zc_t = ctx.enter_context(nc.sbuf_tensor([P, blocks], F32))
        zsum_t = ctx.enter_context(nc.sbuf_tensor([P, 1], F32))
        stage_t = ctx.enter_context(nc.sbuf_tensor([P, blocks * SEGW], BF16))
        nexpw_t = ctx.enter_context(nc.sbuf_tensor("nexpw_t", [P, nch], F32))
        uat = [ctx.enter_context(nc.sbuf_tensor(f"uat{j}", [P, SEGW], BF16))
               for j in range(max(1, blk_ch - kd - kg))]
        atd = [ctx.enter_context(nc.sbuf_tensor(f"atd{j}", [P, SEGW], BF16))
               for j in range(NAT)]
        atg = [ctx.enter_context(nc.sbuf_tensor(f"atg{j}", [P, SEGW], BF16))
               for j in range(NAT)]
        ata = [ctx.enter_context(nc.sbuf_tensor(f"ata{j}", [P, SEGW], BF16))
               for j in range(NAT)]
        pt = [ctx.enter_context(nc.psum_tensor(f"pt{j}", [P, BPB * SEGW], F32))
              for j in range(2)]

        def fw(eng, ins, sem, val):
            # attach a wait: fused onto the instruction (wait queue) or as a
            # separate EventSemaphore on the SEQ, per FUSEW
            if fusew:
                return ins()._wait_ge(sem, val)
            eng.wait_ge(sem, val)
            return ins()

        with nc.Block() as block:

            @block.sync
            def _(sync):
                sync.dma_start(out=iota_t[:], in_=iota_h[:]).then_inc(sem_xc, 16)
                sync.dma_start(out=bl_t[:], in_=bl_h[:]).then_inc(sem_xc, 16)
                for b in range(blocks):
                    j = b % NXB
                    if b >= NXB:
                        bo = b - NXB
                        sync.wait_ge(sem_dve, T_DVE_F1[bo])
                        sync.wait_ge(sem_pe, T_PE[(bo, blk_ch - 1)])
                    sync.dma_start(
                        out=xt[j][:].rearrange("p (c d) -> p c d", d=D),
                        in_=xp_h[b * blk_ch * P * D:(b + 1) * blk_ch * P * D]
                        .rearrange("(p c d) -> p c d", p=P, d=D),
                    ).then_inc(sem_x[j], 16)
                for g in range(ngrp):
                    sync.wait_ge(sem_cp, g + 1)
                    w = min(BPB * SEGW, blocks * SEGW - g * BPB * SEGW)
                    sync.dma_start(
                        out=out_h[:, g * BPB * SEGW:g * BPB * SEGW + w],
                        in_=stage_t[:, g * BPB * SEGW:g * BPB * SEGW + w],
                    ).then_inc(sem_out, 16)
                sync.wait_ge(sem_dve, zred_tick)
                sync.dma_start(out=z_h[:], in_=zsum_t[:]).then_inc(sem_out, 16)
                nout = ngrp + 1
                if dbg:
                    sync.dma_start(out=sc_h[:], in_=scores_t[:]).then_inc(sem_out, 16)
                    sync.dma_start(out=ew_h[:], in_=expw_t[:]).then_inc(sem_out, 16)
                    nout += 2
                sync.wait_ge(sem_out, 16 * nout)

            @block.vector
            def _(vector):
                # bln = -bl (f32, 1x) ; fused wait on prologue DMAs
                fw(vector, lambda: nc.vector.tensor_scalar_mul(
                    bln_t[:], bl_t[:], -1.0,
                ), sem_xc, 32).then_inc(sem_dve, 1)
                for b in range(blocks + 1):
                    if b < blocks:
                        j = b % NXB
                        x3 = xt[j][:].rearrange("p (c d) -> p c d", d=D)
                        w3 = xw_t[:].rearrange("p (c d) -> p c d", d=D // 2)
                        # fold tree 128 -> 8 (2x bf16 TT), then 1x reduce
                        fw(vector, lambda: nc.vector.tensor_tensor(
                            out=w3[:, :, 0:64], in0=x3[:, :, 0:64],
                            in1=x3[:, :, 64:128], op=ALU.add,
                        ), sem_x[j], 16 * (b // NXB + 1)).then_inc(sem_dve, 1)
                        # sem-chain the fold tree: this build overlaps
                        # back-to-back engine instructions, so a same-engine
                        # RAW needs the sem (fires post-drain) to be safe
                        prev = T_DVE_F1[b]
                        for k, w in enumerate((32, 16, 8)):
                            fw(vector, lambda: nc.vector.tensor_tensor(
                                out=w3[:, :, 0:w], in0=w3[:, :, 0:w],
                                in1=w3[:, :, w:2 * w], op=ALU.add,
                            ), sem_dve, prev).then_inc(sem_dve, 1)
                            prev = T_DVE_F[(b, k)]
                        fw(vector, lambda: nc.vector.tensor_reduce(
                            out=scores_t[:, b * blk_ch:(b + 1) * blk_ch],
                            in_=w3[:, :, 0:8],
                            axis=mybir.AxisListType.X, op=ALU.add,
                        ), sem_dve, prev).then_inc(sem_dve, 1)
                    if b >= 1 and kd > 0:
                        bm = b - 1
                        vector.wait_ge(sem_act, T_ACT_EXP[bm])
                        for i in range(kd):
                            gd = bm * kd + i
                            ca = bm * blk_ch + pos_d(i)
                            mk = lambda: nc.vector.tensor_scalar(
                                atd[gd % NAT][:], iota_t[:],
                                bl_t[:, ca:ca + 1], expw_t[:, ca:ca + 1],
                                ALU.is_equal, ALU.mult,
                            )
                            if gd >= NAT:
                                bo, io = divmod(gd - NAT, kd)
                                ins = fw(vector, mk, sem_pe, T_PE[(bo, pos_d(io))])
                            else:
                                ins = mk()
                            ins.then_inc(sem_dve, 1)
                # Z final reduction
                fw(vector, lambda: nc.vector.tensor_reduce(
                    out=zsum_t[:], in_=zc_t[:],
                    axis=mybir.AxisListType.X, op=ALU.add,
                ), sem_act, T_ACT_EXP[blocks - 1]).then_inc(sem_dve, 1)

            @block.gpsimd
            def _(gpsimd):
                gpsimd.wait_ge(sem_xc, 32)
                for b in range(1, blocks + 1):
                    if kg == 0:
                        continue
                    bm = b - 1
                    gpsimd.wait_ge(sem_act, T_ACT_EXP[bm])
                    for i in range(kg):
                        gg = bm * kg + i
                        ca = bm * blk_ch + pos_g(i)
                        mk = lambda: nc.gpsimd.tensor_scalar(
                            atg[gg % NAT][:], iota_t[:],
                            bl_t[:, ca:ca + 1], expw_t[:, ca:ca + 1],
                            ALU.is_equal, ALU.mult,
                        )
                        if gg >= NAT:
                            bo, io = divmod(gg - NAT, kg)
                            ins = fw(gpsimd, mk, sem_pe, T_PE[(bo, pos_g(io))])
                        else:
                            ins = mk()
                        ins.then_inc(sem_gp, 1)

            @block.scalar
            def _(scalar):
                ka_ = ka
                scalar.wait_ge(sem_dve, 1)  # bln ready (covers iota/bl too)
                for b in range(blocks + 1):
                    if b < blocks:
                        fw(scalar, lambda: nc.scalar.activation(
                            out=expw_t[:, b * blk_ch:(b + 1) * blk_ch],
                            in_=scores_t[:, b * blk_ch:(b + 1) * blk_ch],
                            func=ACTF.Exp,
                            accum_out=zc_t[:, b:b + 1],
                        ), sem_dve, T_DVE_RED[b]).then_inc(sem_act, 1)
                        if b == 0 and ka_ > 0:
                            nc.scalar.activation(
                                out=nexpw_t[:, 0:blk_ch],
                                in_=expw_t[:, 0:blk_ch],
                                func=ACTF.Copy, scale=-1.0,
                            )._wait_ge(sem_act, T_ACT_EXP[0])
                    if b >= 1 and ka_ > 0:
                        bm = b - 1
                        # all Squares first, then nexpw, then all Relus: the
                        # >= ka-1 instruction spacing keeps same-engine RAW on
                        # uat[] safe without sems
                        for jx in range(ka_):
                            ca = bm * blk_ch + pos_a(jx)
                            nc.scalar.activation(
                                out=uat[jx][:], in_=iota_t[:], func=ACTF.Square,
                                bias=bln_t[:, ca:ca + 1], scale=1.0,
                            )
                        if b < blocks:
                            nc.scalar.activation(
                                out=nexpw_t[:, b * blk_ch:(b + 1) * blk_ch],
                                in_=expw_t[:, b * blk_ch:(b + 1) * blk_ch],
                                func=ACTF.Copy, scale=-1.0,
                            )._wait_ge(sem_act, T_ACT_EXP[b])
                        for jx in range(ka_):
                            ga = bm * ka_ + jx
                            ca = bm * blk_ch + pos_a(jx)
                            # A = Relu(expw - u*expw), exact at integer iota/bl
                            mk = lambda: nc.scalar.activation(
                                out=ata[ga % NAT][:], in_=uat[jx][:],
                                func=ACTF.Relu,
                                bias=expw_t[:, ca:ca + 1],
                                scale=nexpw_t[:, ca:ca + 1],
                            )
                            if ga >= NAT:
                                bo, jo = divmod(ga - NAT, ka_)
                                ins = fw(scalar, mk, sem_pe, T_PE[(bo, pos_a(jo))])
                            else:
                                ins = mk()
                            ins.then_inc(sem_act, 1)
                    if b >= 1 and ((b - 1) % BPB == BPB - 1 or b - 1 == blocks - 1):
                        lb = b - 1
                        g = lb // BPB
                        nblk = lb - g * BPB + 1
                        fw(scalar, lambda: nc.scalar.copy(
                            out=stage_t[:, g * BPB * SEGW:
                                        g * BPB * SEGW + nblk * SEGW],
                            in_=pt[g % 2][:, 0:nblk * SEGW],
                        ), sem_pe, T_PE[(lb, blk_ch - 1)]).then_inc(sem_cp, 1)

            @block.tensor
            def _(tensor):
                for b in range(blocks):
                    j = b % NXB
                    g = b // BPB
                    tensor.wait_ge(sem_x[j], 16 * (b // NXB + 1))
                    if b % BPB == 0 and g >= 2:
                        tensor.wait_ge(sem_cp, g - 1)
                    off = (b % BPB) * SEGW
                    for c in range(blk_ch):
                        if c < kd:
                            a = atd[(b * kd + c) % NAT]
                            tick = T_DVE_OH[(b, c)]
                            sem = sem_dve
                        elif c < kd + kg:
                            i = c - kd
                            a = atg[(b * kg + i) % NAT]
                            tick = T_GP_OH[(b, i)]
                            sem = sem_gp
                        else:
                            jx = c - kd - kg
                            a = ata[(b * ka + jx) % NAT]
                            tick = T_ACT_OH[(b, jx)]
                            sem = sem_act
                        fw(tensor, lambda: nc.tensor.matmul(
                            pt[g % 2][:, off:off + SEGW],
                            lhsT=xt[j][:, c * D:(c + 1) * D],
                            rhs=a[:],
                            start=(c == 0),
                            stop=(c == blk_ch - 1),
                        ), sem, tick).then_inc(sem_pe, 1)

    return nc


def _pool(x, batch, W, num_graphs, n_cores=N_CORES, kd=None, kg=None):
    segs_per_core = num_graphs // n_cores
    blocks = segs_per_core // SEGW

    seg_starts = np.searchsorted(batch, np.arange(0, num_graphs + 1, SEGW))
    blk_cnt = np.diff(seg_starts)
    blk_ch = max(1, int(np.ceil(blk_cnt.max() / P)))    # chunks per block
    n_b = blk_ch * P
    nch = blocks * blk_ch
    L = blocks * n_b
    if kd is None:
        kd = int(os.environ.get("KD", "5"))
    if kg is None:
        kg = int(os.environ.get("KG", "22"))
    kd = min(kd, blk_ch)
    kg = min(kg, blk_ch - kd)

    Wv = np.asarray(W, np.float32).reshape(D)
    xw_bf = (np.asarray(x, np.float32) * Wv[None, :]).astype(ml_dtypes.bfloat16)
    bloc_all = (batch % SEGW).astype(np.float32)

    iota = np.broadcast_to(np.arange(SEGW, dtype=np.float32), (P, SEGW)).astype(
        ml_dtypes.bfloat16)

    in_maps, pad_counts = [], []
    for core in range(n_cores):
        xflat = np.zeros((L, D), ml_dtypes.bfloat16)
        blflat = np.full((L,), 999.0, np.float32)
        for bi in range(blocks):
            gb = core * blocks + bi
            s0, s1 = seg_starts[gb], seg_starts[gb + 1]
            cnt = s1 - s0
            xflat[bi * n_b: bi * n_b + cnt] = xw_bf[s0:s1]
            blflat[bi * n_b: bi * n_b + cnt] = bloc_all[s0:s1]
        # per-BLOCK slabs of [P, blk_ch, D] (each block's DMA reads one slab)
        xp = np.ascontiguousarray(
            xflat.reshape(blocks, blk_ch, P, D).transpose(0, 2, 1, 3)
        ).reshape(-1)
        bl = np.ascontiguousarray(blflat.reshape(nch, P).T)
        pad_counts.append(L - int(blk_cnt[core * blocks:(core + 1) * blocks].sum()))
        in_maps.append({"xp": xp, "bl": bl, "iota": iota})

    key = (blocks, blk_ch, kd, kg, os.environ.get("FUSEW", "1"))
    if key not in _prog_cache:
        _prog_cache[key] = _build(blocks, blk_ch, kd, kg)
    nc = _prog_cache[key]

    res = run_bass_kernel_spmd(nc, in_maps, list(range(n_cores))).results

    z_total = 0.0
    parts = []
    for core in range(n_cores):
        z_total += float(res[core]["zout"].astype(np.float64).sum()) - pad_counts[core]
        o = res[core]["outp"].astype(np.float32)     # [D, segs_per_core]
        parts.append((o / Wv[:, None]).T)            # [segs_per_core, D]
    out = np.concatenate(parts, axis=0)
    return (out / np.float32(z_total)).astype(np.float32)


def kernel(x, batch, W, b):
    x = np.asarray(x, np.float32)
    batch = np.asarray(batch)
    W = np.asarray(W, np.float32)
    return _pool(x, batch, W, num_graphs=16384)


if __name__ == "__main__":
    rng = np.random.default_rng(0)
    G = 1024
    n = 16000
    x = rng.standard_normal((n, D), dtype=np.float32)
    batch = np.sort(rng.integers(0, G, n)).astype(np.int64)
    W = (rng.standard_normal((D, 1), dtype=np.float32) / np.sqrt(D)).astype(np.float32)
    b = np.zeros((1,), np.float32)

    got = _pool(x, batch, W, num_graphs=G)

    s = (x @ W).ravel()
    a = np.exp(s - s.max()); a /= a.sum()
    want = np.zeros((G, D), np.float64)
    np.add.at(want, batch, x * a[:, None])
    want = want.astype(np.float32)
    num = np.abs(got - want).max()
    print("abs err:", num, "rel err:", num / np.abs(want).max())


# revision 6
# speedup vs baseline: 1.0852x; 1.0852x over previous
"""AttentionPooling (global-softmax segment-sum) Trainium2 Bass kernel, v2.

  scores = x @ W + b ; attn = softmax(scores, axis=0) ; out = segment_sum(x*attn, batch, G)

Design (8 cores, SPMD, raw Bass; one fused semaphore wait per instruction):

 * softmax shift-invariant => b drops out; fixed shift 0 (scores ~ N(0,1)).
 * host streams xw = x * W^T (bf16). Device computes per core the
   unnormalized pooledT'[d, g] = sum_{i in g} e^{s_i} xw[i, d] and
   Z_core = sum_i e^{s_i}; host divides by W[d] (undoes the fold) and by
   Z = sum Z_core. scores s_i = sum_d xw[i, d] (device fold tree + reduce).
 * shard by SEGMENT BLOCKS of SEGW=32 segs: G -> cores x blocks x 32. batch
   sorted => block nodes contiguous; host pads every block to blk_ch chunks
   of 128 nodes. Pad nodes: xw=0 => e^0=1 pollutes only Z (host subtracts
   the pad count); pad batchloc=999 never matches any one-hot column.
 * per block b (one x-tile [128, blk_ch x 128] bf16):
     SYNC dma xt[b%NXB]
     DVE  fold tree d:128->64->32->16->8 (bf16 TT, 2x mode) into xw scratch,
          then 1x tensor_reduce -> scores[:, blk] f32
     ACT  expw = Exp(scores) with accum_out -> zc[:, b]
     one-hot A[p, j] = (iota_j == batchloc_p) * expw_p  [128 nodes, 32 segs],
     built per chunk, split kd on DVE (TS is_eq+mult, 4x, ~70ns), kg on Pool
     (~139ns), ka on ACT (Square(iota+bln) then Exp(-100*u + s), exact at
     integer iota/bl; ~424ns)
     PE   psumT[128 d, 32 segs] += xt_chunk.T @ A  (lhsT=xt chunk, rhs=A;
          out free = 32 => 13-27ns/chunk at full speed)
   one-hot work of block b-1 overlaps scores of block b (software pipeline).
 * psum: 2 banks [128, 512]; 16 blocks (512 segs) per bank; ACT copies each
   full bank -> bf16 stage; one DMA out per bank group.
 * waits are FUSED onto compute instructions (wait queue, not SEQ) wherever
   only one condition is needed; separate wait_ge otherwise.
"""

import os
import numpy as np
import ml_dtypes

import concourse.bass as bass
import concourse.mybir as mybir
from concourse.bass_utils import run_bass_kernel_spmd

BF16 = mybir.dt.bfloat16
F32 = mybir.dt.float32
ALU = mybir.AluOpType
ACTF = mybir.ActivationFunctionType

N_CORES = 8
D = 128
P = 128
SEGW = 32              # segments per block (= one-hot width = psum slice)
BPB = 16               # blocks per psum bank group (16*32 = 512 f32 cols)
NXB = 12               # x-tile buffer depth
# one-hot tile slots per engine = 2 blocks worth (reuse reaches 2 back)
BIGM = -100.0          # ACT one-hot: A = Exp(BIGM*u + s), u=(iota-bl)^2

_prog_cache = {}


def _build(blocks, bc, kd, ka):
    """blocks SEGW-seg blocks/core; bc[b] chunks (of 128 nodes) per block
    (varies by block index: max over cores); one-hot split per block: kd on
    DVE, ka on ACT (fixed), the rest (kg_of[b]) on Pool."""
    bc = list(bc)
    assert len(bc) == blocks
    kg_of = [c - kd - ka for c in bc]
    assert min(kg_of) >= 0
    blk_ch = max(bc)
    CH0 = [0]
    for c in bc:
        CH0.append(CH0[-1] + c)
    KG0 = [0]
    for g in kg_of:
        KG0.append(KG0[-1] + g)
    import bisect
    natd, natg, nata = 2 * kd, 2 * max(1, max(kg_of)), 2 * ka
    fusew = int(os.environ.get("FUSEW", "1"))
    nch = CH0[blocks]
    ngrp = (blocks + BPB - 1) // BPB

    nc = bass.Bass()

    xp_h = nc.declare_dram_parameter("xp", [nch * P * D], BF16, isOutput=False)
    bl_h = nc.declare_dram_parameter("bl", [P, nch], F32, isOutput=False)
    iota_h = nc.declare_dram_parameter("iota", [P, SEGW], BF16, isOutput=False)
    out_h = nc.declare_dram_parameter("outp", [P, blocks * SEGW], BF16, isOutput=True)
    z_h = nc.declare_dram_parameter("zout", [P, 2], F32, isOutput=True)
    dbg = int(os.environ.get("DBG", "0"))
    if dbg:
        sc_h = nc.declare_dram_parameter("scdbg", [P, nch], F32, isOutput=True)
        ew_h = nc.declare_dram_parameter("ewdbg", [P, nch], F32, isOutput=True)

    # tick tables (pure counting in emission order) --------------------------
    FILL_ITERS = 3

    def dve_ops(it):
        """DVE per-iter op list: ("f1",bf) / ("f2".."red",bm) / ("oh",bo,i) /
        ("bln",) / ("xcwait",) / ("cover",). During fill, folds precede f1 so
        scores aren't gated on the next x tile."""
        bf, bm, bo = it, it - 1, it - 2
        bm_ok = 0 <= bm < blocks
        bo_ok = 0 <= bo < blocks
        seq = []
        if bo_ok and kd > 0:
            seq.append(("cover",))
        if it == 2:
            seq.append(("xcwait",))
        fill = it <= FILL_ITERS
        if not fill and bf < blocks:
            seq.append(("f1", bf))
        if bm_ok:
            seq.append(("f2", bm))
        if bo_ok:
            seq.append(("oh", bo, 0))
        if bm_ok:
            seq.append(("f3", bm))
            if bo_ok and kd > 1:
                seq.append(("oh", bo, 1))
            seq.append(("f4", bm))
            if bo_ok and kd > 2:
                seq.append(("oh", bo, 2))
            seq.append(("red", bm))
        if bo_ok:
            for i in range((3 if bm_ok else 1), kd):
                seq.append(("oh", bo, i))
        if it == 1:
            seq.append(("bln",))
        if fill and bf < blocks:
            seq.append(("f1", bf))
        return seq

    T_DVE_F1, T_DVE_FOLD, T_DVE_RED, T_DVE_OH = {}, {}, {}, {}
    bln_tick = None
    t = 0
    for it in range(blocks + 2):
        for op in dve_ops(it):
            if op[0] in ("cover", "xcwait"):
                continue
            t += 1
            if op[0] == "f1":
                T_DVE_F1[op[1]] = t
            elif op[0] == "oh":
                T_DVE_OH[(op[1], op[2])] = t
            elif op[0] == "red":
                T_DVE_RED[op[1]] = t
                T_DVE_FOLD[(op[1], "red")] = t
            elif op[0] == "bln":
                bln_tick = t
            else:
                T_DVE_FOLD[(op[1], op[0])] = t
    zred_tick = t + 2
    # Pool: per iter (2..blocks+1): kg_of[bo] one-hots of bo = it-2
    T_GP_OH = {}
    t = 0
    for it in range(2, blocks + 2):
        for i in range(kg_of[it - 2]):
            t += 1; T_GP_OH[(it - 2, i)] = t
    # ACT per iter: Exp(bm)(+1), [ka Squares(bo)], nexpw(bm), [ka Relus(bo)
    # (+1 each)], bank copy
    T_ACT_EXP, T_ACT_OH = {}, {}
    t = 0
    for it in range(blocks + 3):
        bm, bo = it - 1, it - 2
        if 0 <= bm < blocks:
            t += 1; T_ACT_EXP[bm] = t
        if 0 <= bo < blocks:
            for j in range(ka):
                t += 1; T_ACT_OH[(bo, j)] = t
    # PE: one mm per chunk
    T_PE = {}
    t = 0
    for b in range(blocks):
        for c in range(bc[b]):
            t += 1; T_PE[(b, c)] = t

    # chunk position of each engine's i-th one-hot within a block
    def pos_d(i):
        return i

    def pos_g(i):
        return kd + i

    def pos_a_of(b, j):
        return kd + kg_of[b] + j

    import contextlib
    with contextlib.ExitStack() as ctx:
        sem_xc = ctx.enter_context(nc.semaphore("sem_xc"))
        sem_x = [ctx.enter_context(nc.semaphore(f"sem_x{j}")) for j in range(NXB)]
        sem_dve = ctx.enter_context(nc.semaphore("sem_dve"))
        sem_act = ctx.enter_context(nc.semaphore("sem_act"))
        sem_gp = ctx.enter_context(nc.semaphore("sem_gp"))
        sem_pe = ctx.enter_context(nc.semaphore("sem_pe"))
        sem_cp = ctx.enter_context(nc.semaphore("sem_cp"))
        sem_out = ctx.enter_context(nc.semaphore("sem_out"))

        iota_t = ctx.enter_context(nc.sbuf_tensor([P, SEGW], BF16))
        bl_t = ctx.enter_context(nc.sbuf_tensor([P, nch], F32))
        bln_t = ctx.enter_context(nc.sbuf_tensor([P, nch], F32))
        xt = [ctx.enter_context(nc.sbuf_tensor(f"xt{j}", [P, blk_ch * D], BF16))
              for j in range(NXB)]
        xw1 = [ctx.enter_context(nc.sbuf_tensor(f"xw1_{j}", [P, blk_ch * 64], BF16))
               for j in range(2)]
        xw2 = ctx.enter_context(nc.sbuf_tensor("xw2", [P, blk_ch * 32], BF16))
        xw3 = ctx.enter_context(nc.sbuf_tensor("xw3", [P, blk_ch * 16], BF16))
        xw4 = ctx.enter_context(nc.sbuf_tensor("xw4", [P, blk_ch * 8], BF16))
        scores_t = ctx.enter_context(nc.sbuf_tensor([P, nch], F32))
        expw_t = ctx.enter_context(nc.sbuf_tensor([P, nch], F32))
        zsum_t = ctx.enter_context(nc.sbuf_tensor([P, 2], F32))
        stage_t = ctx.enter_context(nc.sbuf_tensor([P, blocks * SEGW], BF16))
        nexpw_t = ctx.enter_context(nc.sbuf_tensor("nexpw_t", [P, nch], F32))
        uat = [ctx.enter_context(nc.sbuf_tensor(f"uat{j}", [P, SEGW], BF16))
               for j in range(max(1, ka))]
        atd = [ctx.enter_context(nc.sbuf_tensor(f"atd{j}", [P, SEGW], BF16))
               for j in range(max(1, natd))]
        atg = [ctx.enter_context(nc.sbuf_tensor(f"atg{j}", [P, SEGW], BF16))
               for j in range(max(1, natg))]
        ata = [ctx.enter_context(nc.sbuf_tensor(f"ata{j}", [P, SEGW], BF16))
               for j in range(max(1, nata))]
        pt = [ctx.enter_context(nc.psum_tensor(f"pt{j}", [P, BPB * SEGW], F32))
              for j in range(2)]

        def fw(eng, ins, sem, val):
            # attach a wait: fused onto the instruction (wait queue) or as a
            # separate EventSemaphore on the SEQ, per FUSEW
            if fusew:
                return ins()._wait_ge(sem, val)
            eng.wait_ge(sem, val)
            return ins()

        with nc.Block() as block:

            @block.sync
            def _(sync):
                sync.dma_start(out=iota_t[:], in_=iota_h[:]).then_inc(sem_xc, 16)
                for b in range(blocks):
                    j = b % NXB
                    if b == 1:
                        sync.dma_start(out=bl_t[:], in_=bl_h[:]).then_inc(sem_xc, 16)
                    if b >= NXB:
                        bo = b - NXB
                        sync.wait_ge(sem_dve, T_DVE_F1[bo])
                        sync.wait_ge(sem_pe, T_PE[(bo, bc[bo] - 1)])
                    sync.dma_start(
                        out=xt[j][:, 0:bc[b] * D].rearrange(
                            "p (c d) -> p c d", d=D),
                        in_=xp_h[CH0[b] * P * D:CH0[b + 1] * P * D]
                        .rearrange("(p c d) -> p c d", p=P, d=D),
                    ).then_inc(sem_x[j], 16)
                if blocks <= 1:
                    sync.dma_start(out=bl_t[:], in_=bl_h[:]).then_inc(sem_xc, 16)
                for g in range(ngrp):
                    sync.wait_ge(sem_cp, g + 1)
                    w = min(BPB * SEGW, blocks * SEGW - g * BPB * SEGW)
                    sync.dma_start(
                        out=out_h[:, g * BPB * SEGW:g * BPB * SEGW + w],
                        in_=stage_t[:, g * BPB * SEGW:g * BPB * SEGW + w],
                    ).then_inc(sem_out, 16)
                sync.wait_ge(sem_dve, zred_tick)
                sync.dma_start(out=z_h[:], in_=zsum_t[:]).then_inc(sem_out, 16)
                sync.wait_ge(sem_out, 16 * (ngrp + 1))

            @block.vector
            def _(vector):
                prev_fold_tick = {}
                last_was_fold = {}
                for it in range(blocks + 2):
                    for op in dve_ops(it):
                        kind = op[0]
                        if kind == "cover":
                            bo = it - 2
                            gd_last = bo * kd + kd - 1
                            if gd_last >= natd:
                                po, io = divmod(gd_last - natd, kd)
                                vector.wait_ge(sem_pe, T_PE[(po, pos_d(io))])
                            continue
                        if kind == "xcwait":
                            vector.wait_ge(sem_xc, 32)
                            continue
                        if kind == "bln":
                            fw(vector, lambda: nc.vector.tensor_scalar_mul(
                                bln_t[:], bl_t[:], -1.0,
                            ), sem_xc, 32).then_inc(sem_dve, 1)
                            continue
                        if kind == "f1":
                            bf = op[1]
                            j = bf % NXB
                            nb = bc[bf]
                            x3 = xt[j][:, 0:nb * D].rearrange(
                                "p (c d) -> p c d", d=D)
                            w1 = xw1[bf % 2][:, 0:nb * 64].rearrange(
                                "p (c d) -> p c d", d=64)
                            fw(vector, lambda: nc.vector.tensor_tensor(
                                out=w1[:, :, 0:64], in0=x3[:, :, 0:64],
                                in1=x3[:, :, 64:128], op=ALU.add,
                            ), sem_x[j], 16 * (bf // NXB + 1)).then_inc(sem_dve, 1)
                            last_was_fold[op[1]] = False
                            continue
                        if kind == "oh":
                            bo, i = op[1], op[2]
                            ca = CH0[bo] + pos_d(i)
                            gd = bo * kd + i
                            mk = lambda: nc.vector.tensor_scalar(
                                atd[gd % natd][:], iota_t[:],
                                bl_t[:, ca:ca + 1], expw_t[:, ca:ca + 1],
                                ALU.is_equal, ALU.mult,
                            )
                            if i == 0:
                                ins = fw(vector, mk, sem_act, T_ACT_EXP[bo])
                            else:
                                ins = mk()
                            ins.then_inc(sem_dve, 1)
                            last_was_fold[it - 1] = False
                            continue
                        # fold ops f2/f3/f4/red of block bm
                        bm = op[1]
                        nb = bc[bm]
                        w2 = xw1[bm % 2][:, 0:nb * 64].rearrange(
                            "p (c d) -> p c d", d=64)
                        w3 = xw2[:, 0:nb * 32].rearrange("p (c d) -> p c d", d=32)
                        w4 = xw3[:, 0:nb * 16].rearrange("p (c d) -> p c d", d=16)
                        w5 = xw4[:, 0:nb * 8].rearrange("p (c d) -> p c d", d=8)
                        if kind == "f2":
                            mk = lambda: nc.vector.tensor_tensor(
                                out=w3[:, :, 0:32], in0=w2[:, :, 0:32],
                                in1=w2[:, :, 32:64], op=ALU.add)
                            # during fill, f1(bm) may be the immediately
                            # preceding DVE instruction
                            need_chain = it <= FILL_ITERS + 1
                            prev = T_DVE_F1[bm]
                        else:
                            if kind == "f3":
                                mk = lambda: nc.vector.tensor_tensor(
                                    out=w4[:, :, 0:16], in0=w3[:, :, 0:16],
                                    in1=w3[:, :, 16:32], op=ALU.add)
                            elif kind == "f4":
                                mk = lambda: nc.vector.tensor_tensor(
                                    out=w5[:, :, 0:8], in0=w4[:, :, 0:8],
                                    in1=w4[:, :, 8:16], op=ALU.add)
                            else:
                                mk = lambda: nc.vector.tensor_reduce(
                                    out=scores_t[:, CH0[bm]:CH0[bm + 1]],
                                    in_=w5[:, :, 0:8],
                                    axis=mybir.AxisListType.X, op=ALU.add)
                            need_chain = last_was_fold.get(bm, True)
                            prev = prev_fold_tick.get(bm)
                        if need_chain and prev is not None:
                            ins = fw(vector, mk, sem_dve, prev)
                        else:
                            ins = mk()
                        ins.then_inc(sem_dve, 1)
                        prev_fold_tick[bm] = T_DVE_FOLD[(bm, kind)]
                        last_was_fold[bm] = True
                # Z split: bulk over blocks 0..n-2 early, last block late
                cut = CH0[blocks - 1]
                fw(vector, lambda: nc.vector.tensor_reduce(
                    out=zsum_t[:, 0:1], in_=expw_t[:, 0:cut],
                    axis=mybir.AxisListType.X, op=ALU.add,
                ), sem_act, T_ACT_EXP[max(0, blocks - 2)]).then_inc(sem_dve, 1)
                fw(vector, lambda: nc.vector.tensor_reduce(
                    out=zsum_t[:, 1:2], in_=expw_t[:, cut:nch],
                    axis=mybir.AxisListType.X, op=ALU.add,
                ), sem_act, T_ACT_EXP[blocks - 1]).then_inc(sem_dve, 1)

            @block.gpsimd
            def _(gpsimd):
                if max(kg_of) > 0:
                    gpsimd.wait_ge(sem_xc, 32)
                    for it in range(2, blocks + 2):
                        bo = it - 2
                        if kg_of[bo] == 0:
                            continue
                        gg_last = KG0[bo] + kg_of[bo] - 1
                        if gg_last >= natg:
                            r = gg_last - natg
                            pb = bisect.bisect_right(KG0, r) - 1
                            gpsimd.wait_ge(sem_pe, T_PE[(pb, pos_g(r - KG0[pb]))])
                        gpsimd.wait_ge(sem_act, T_ACT_EXP[bo])
                        for i in range(kg_of[bo]):
                            ca = CH0[bo] + pos_g(i)
                            gg = KG0[bo] + i
                            nc.gpsimd.tensor_scalar(
                                atg[gg % natg][:], iota_t[:],
                                bl_t[:, ca:ca + 1], expw_t[:, ca:ca + 1],
                                ALU.is_equal, ALU.mult,
                            ).then_inc(sem_gp, 1)

            @block.scalar
            def _(scalar):
                ka_ = ka
                for it in range(blocks + 3):
                    bm = it - 1       # exp/nexpw/Square block
                    bo = it - 2       # Relu block
                    if 0 <= bm < blocks:
                        fw(scalar, lambda: nc.scalar.activation(
                            out=expw_t[:, CH0[bm]:CH0[bm + 1]],
                            in_=scores_t[:, CH0[bm]:CH0[bm + 1]],
                            func=ACTF.Exp,
                        ), sem_dve, T_DVE_RED[bm]).then_inc(sem_act, 1)
                    if 0 <= bo < blocks and ka_ > 0:
                        for jx in range(ka_):
                            ca = CH0[bo] + pos_a_of(bo, jx)
                            mk = lambda: nc.scalar.activation(
                                out=uat[jx][:], in_=iota_t[:],
                                func=ACTF.Square,
                                bias=bln_t[:, ca:ca + 1], scale=1.0,
                            )
                            if jx == 0:
                                # bln ready (DVE); also covers iota/bl
                                ins = fw(scalar, mk, sem_dve, bln_tick)
                            else:
                                ins = mk()
                    if 0 <= bm < blocks and ka_ > 0:
                        # nexpw for bm; spaced from exp(bm) by the Squares
                        # above when present, else self-sem spaced
                        ins = nc.scalar.activation(
                            out=nexpw_t[:, CH0[bm]:CH0[bm + 1]],
                            in_=expw_t[:, CH0[bm]:CH0[bm + 1]],
                            func=ACTF.Copy, scale=-1.0,
                        )
                        if not (0 <= bo < blocks):
                            ins._wait_ge(sem_act, T_ACT_EXP[bm])
                    if 0 <= bo < blocks and ka_ > 0:
                        ga_last = bo * ka_ + ka_ - 1
                        if ga_last >= nata:
                            po, jo = divmod(ga_last - nata, ka_)
                            scalar.wait_ge(sem_pe, T_PE[(po, pos_a_of(po, jo))])
                        for jx in range(ka_):
                            ga = bo * ka_ + jx
                            ca = CH0[bo] + pos_a_of(bo, jx)
                            # A = Relu(expw - u*expw), exact at integer iota/bl
                            nc.scalar.activation(
                                out=ata[ga % nata][:], in_=uat[jx][:],
                                func=ACTF.Relu,
                                bias=expw_t[:, ca:ca + 1],
                                scale=nexpw_t[:, ca:ca + 1],
                            ).then_inc(sem_act, 1)
                    bcp = it - 3      # bank-copy trigger block
                    if 0 <= bcp < blocks and (bcp % BPB == BPB - 1
                                              or bcp == blocks - 1):
                        g = bcp // BPB
                        nblk = bcp - g * BPB + 1
                        fw(scalar, lambda: nc.scalar.copy(
                            out=stage_t[:, g * BPB * SEGW:
                                        g * BPB * SEGW + nblk * SEGW],
                            in_=pt[g % 2][:, 0:nblk * SEGW],
                        ), sem_pe, T_PE[(bcp, bc[bcp] - 1)]).then_inc(sem_cp, 1)

            @block.tensor
            def _(tensor):
                for b in range(blocks):
                    j = b % NXB
                    g = b // BPB
                    tensor.wait_ge(sem_x[j], 16 * (b // NXB + 1))
                    if b % BPB == 0 and g >= 2:
                        tensor.wait_ge(sem_cp, g - 1)
                    off = (b % BPB) * SEGW
                    for c in range(bc[b]):
                        if c < kd:
                            a = atd[(b * kd + c) % natd]
                            tick = T_DVE_OH[(b, c)]
                            sem = sem_dve
                        elif c < kd + kg_of[b]:
                            i = c - kd
                            a = atg[(KG0[b] + i) % natg]
                            tick = T_GP_OH[(b, i)]
                            sem = sem_gp
                        else:
                            jx = c - kd - kg_of[b]
                            a = ata[(b * ka + jx) % nata]
                            tick = T_ACT_OH[(b, jx)]
                            sem = sem_act
                        fw(tensor, lambda: nc.tensor.matmul(
                            pt[g % 2][:, off:off + SEGW],
                            lhsT=xt[j][:, c * D:(c + 1) * D],
                            rhs=a[:],
                            start=(c == 0),
                            stop=(c == bc[b] - 1),
                        ), sem, tick).then_inc(sem_pe, 1)

    return nc


def _pool(x, batch, W, num_graphs, n_cores=N_CORES, kd=None, ka=None):
    segs_per_core = num_graphs // n_cores
    blocks = segs_per_core // SEGW

    seg_starts = np.searchsorted(batch, np.arange(0, num_graphs + 1, SEGW))
    blk_cnt = np.diff(seg_starts).reshape(n_cores, blocks)
    # per-block-index chunk budget: max over cores (SPMD shares one program)
    bc = np.maximum(1, np.ceil(blk_cnt.max(axis=0) / P).astype(int))
    if kd is None:
        kd = int(os.environ.get("KD", "5"))
    if ka is None:
        ka = int(os.environ.get("KA", "6"))
    kd = min(kd, int(bc.min()))
    ka = max(0, min(ka, int(bc.min()) - kd))
    bc = np.maximum(bc, kd + ka)
    CH0 = np.concatenate([[0], np.cumsum(bc)])
    nch = int(CH0[-1])
    L = nch * P

    Wv = np.asarray(W, np.float32).reshape(D)
    xw_bf = (np.asarray(x, np.float32) * Wv[None, :]).astype(ml_dtypes.bfloat16)
    bloc_all = (batch % SEGW).astype(np.float32)

    iota = np.broadcast_to(np.arange(SEGW, dtype=np.float32), (P, SEGW)).astype(
        ml_dtypes.bfloat16)

    in_maps, pad_counts = [], []
    for core in range(n_cores):
        xflat = np.zeros((L, D), ml_dtypes.bfloat16)
        blflat = np.full((L,), 999.0, np.float32)
        for bi in range(blocks):
            gb = core * blocks + bi
            s0, s1 = seg_starts[gb], seg_starts[gb + 1]
            cnt = s1 - s0
            o = CH0[bi] * P
            xflat[o: o + cnt] = xw_bf[s0:s1]
            blflat[o: o + cnt] = bloc_all[s0:s1]
        # per-BLOCK slabs of [P, bc[b], D] (each block's DMA reads one slab)
        slabs = []
        for bi in range(blocks):
            o = CH0[bi] * P
            slabs.append(np.ascontiguousarray(
                xflat[o: o + bc[bi] * P].reshape(bc[bi], P, D)
                .transpose(1, 0, 2)).reshape(-1))
        xp = np.concatenate(slabs)
        bl = np.ascontiguousarray(blflat.reshape(nch, P).T)
        pad_counts.append(L - int(blk_cnt[core].sum()))
        in_maps.append({"xp": xp, "bl": bl, "iota": iota})

    key = (blocks, tuple(int(c) for c in bc), kd, ka,
           os.environ.get("FUSEW", "1"), BPB)
    if key not in _prog_cache:
        _prog_cache[key] = _build(blocks, [int(c) for c in bc], kd, ka)
    nc = _prog_cache[key]

    res = run_bass_kernel_spmd(nc, in_maps, list(range(n_cores))).results

    z_total = 0.0
    parts = []
    for core in range(n_cores):
        z_total += float(res[core]["zout"].astype(np.float64).sum()) - pad_counts[core]
        o = res[core]["outp"].astype(np.float32)     # [D, segs_per_core]
        parts.append((o / Wv[:, None]).T)            # [segs_per_core, D]
    out = np.concatenate(parts, axis=0)
    return (out / np.float32(z_total)).astype(np.float32)


def kernel(x, batch, W, b):
    x = np.asarray(x, np.float32)
    batch = np.asarray(batch)
    W = np.asarray(W, np.float32)
    return _pool(x, batch, W, num_graphs=16384)


if __name__ == "__main__":
    rng = np.random.default_rng(0)
    G = 1024
    n = 16000
    x = rng.standard_normal((n, D), dtype=np.float32)
    batch = np.sort(rng.integers(0, G, n)).astype(np.int64)
    W = (rng.standard_normal((D, 1), dtype=np.float32) / np.sqrt(D)).astype(np.float32)
    b = np.zeros((1,), np.float32)

    got = _pool(x, batch, W, num_graphs=G)

    s = (x @ W).ravel()
    a = np.exp(s - s.max()); a /= a.sum()
    want = np.zeros((G, D), np.float64)
    np.add.at(want, batch, x * a[:, None])
    want = want.astype(np.float32)
    num = np.abs(got - want).max()
    print("abs err:", num, "rel err:", num / np.abs(want).max())
